# revision 26
# baseline (speedup 1.0000x reference)
"""Trainium2 Bass kernel for nn_ExpertNet_GRU (2-layer GRU encoder -> soft
cluster routing -> 8 expert MLPs -> q-weighted combine).

Sharding: data-parallel over batch B=1024 across 8 cores (128 rows/core).
GRU weights + expert weights replicated; no collectives. Each core computes
its own [128, 2] slice of preds; host concatenates.

Layout ("option A", fully transposed): activations live as [feature-on-
partition, batch-on-free] fp16 tiles; weight tiles are stationary lhsT
[K=128, M=128] slices of W.T; gate chunks are folded along the free dim so
pointwise ops run as single [128, 512]/[128, 256] instructions.
"""

import os
import sys

import numpy as np

sys.path.insert(0, "/opt/trn_rl_repo")

import concourse.bass as bass  # noqa: E402
import concourse.tile as tile  # noqa: E402
from concourse import mybir  # noqa: E402
from concourse.bass_utils import run_bass_kernel_spmd  # noqa: E402
from concourse.masks import make_identity  # noqa: E402

B, T, I, H, K = 1024, 128, 128, 256, 8
E1, E2, C = 512, 256, 2
NCORES = 8
BC = B // NCORES  # 128 batch rows per core
XCHUNK = 16  # timesteps per x DMA chunk

f16 = mybir.dt.float16
f32 = mybir.dt.float32
AF = mybir.ActivationFunctionType

_HOIST_UID = [0]


def _hoist_excess_waits(nc):
    """walrus (neuronxcc) in this container allows very few sync-wait slots
    per compute instruction (1 for TT/ACT/TensorScalar). Tile attaches up to
    ~5. Hoist the excess onto standalone InstEventSemaphore entries directly
    before the instruction on the same engine queue — semantically identical
    for monotonic sem-ge waits (engine blocks at the hoisted wait first)."""
    for fn in nc.m.functions:
        for blk in fn.blocks:
            il = blk.instructions
            out = []
            changed = False
            for ins in il:
                keep = 1
                si = ins.sync_info
                if si is not None and si.on_wait and len(si.on_wait) > keep:
                    upd_ids = {u.id for u in si.on_update}
                    waits = list(si.on_wait)
                    hoistable = [
                        w for w in waits
                        if w.sync_type == "semaphore"
                        and w.wait_mode == "sem-ge-imm"
                        and w.wait_reg is None
                        and w.id not in upd_ids
                    ]
                    n_excess = len(waits) - keep
                    excess = hoistable[:n_excess]
                    if excess:
                        kept = [w for w in waits if w not in excess]
                        for w in excess:
                            h = mybir.InstEventSemaphore(
                                name=f"hoistw-{_HOIST_UID[0]}"
                            )
                            _HOIST_UID[0] += 1
                            h.engine = ins.engine
                            h.sync_info = mybir.SyncInfo(
                                on_wait=[w], on_update=[]
                            )
                            out.append(h)
                        ins.sync_info = mybir.SyncInfo(
                            on_wait=kept, on_update=list(si.on_update)
                        )
                        changed = True
                out.append(ins)
            if changed:
                il[:] = out
    return nc


_NC_CACHE = {}
LAST_RESULTS = None


def _build(has_bias: bool, t_steps: int = T):
    nc = bass.Bass()
    tsteps = t_steps

    # ---- DRAM I/O (per core) ----
    xT_d = nc.dram_tensor("xT", [tsteps, I, BC], f16, kind="ExternalInput")
    wT0_d = nc.dram_tensor("wT0", [H + I, 3 * H], f16, kind="ExternalInput")
    wT1_d = nc.dram_tensor("wT1", [2 * H, 3 * H], f16, kind="ExternalInput")
    w1T_d = nc.dram_tensor("w1T", [K, H, E1], f16, kind="ExternalInput")
    w2T_d = nc.dram_tensor("w2T", [K, E1, E2], f16, kind="ExternalInput")
    w3T_d = nc.dram_tensor("w3T", [K, E2, C], f16, kind="ExternalInput")
    cm2T_d = nc.dram_tensor("cm2T", [H, K], f16, kind="ExternalInput")
    cc_d = nc.dram_tensor("cc", [K, H], f32, kind="ExternalInput")
    eb3_d = nc.dram_tensor("eb3", [1, K, C], f16, kind="ExternalInput")
    if has_bias:
        brz0_d = nc.dram_tensor("brz0", [1, 4, 128], f16, kind="ExternalInput")
        bghn0_d = nc.dram_tensor("bghn0", [1, 2, 128], f16, kind="ExternalInput")
        bgin0_d = nc.dram_tensor("bgin0", [1, 2, 128], f16, kind="ExternalInput")
        brz1_d = nc.dram_tensor("brz1", [1, 4, 128], f16, kind="ExternalInput")
        bghn1_d = nc.dram_tensor("bghn1", [1, 2, 128], f16, kind="ExternalInput")
        bgin1_d = nc.dram_tensor("bgin1", [1, 2, 128], f16, kind="ExternalInput")
        eb1T_d = nc.dram_tensor("eb1T", [128, K, 4], f32, kind="ExternalInput")
        eb2T_d = nc.dram_tensor("eb2T", [128, K, 2], f32, kind="ExternalInput")
    preds_d = nc.dram_tensor("preds", [BC, C], f32, kind="ExternalOutput")

    with tile.TileContext(nc) as tc:
        with (
            tc.tile_pool(name="wpool", bufs=1) as wpool,
            tc.tile_pool(name="xpool", bufs=2) as xpool,
            tc.tile_pool(name="hpool", bufs=3) as hpool,
            tc.tile_pool(name="gpool", bufs=2) as gpool,
            tc.tile_pool(name="psum", bufs=1, space="PSUM") as psum,
        ):
            # ---- load constants ----
            wT0 = wpool.tile([128, 3, 3 * H], f16)  # [p, kchunk, gates]
            nc.sync.dma_start(wT0, wT0_d.rearrange("(k p) g -> p k g", p=128))
            wT1 = wpool.tile([128, 4, 3 * H], f16)
            nc.sync.dma_start(wT1, wT1_d.rearrange("(k p) g -> p k g", p=128))
            if has_bias:
                brz0 = wpool.tile([1, 4, 128], f16)
                nc.sync.dma_start(brz0, brz0_d[:, :, :])
                bghn0 = wpool.tile([1, 2, 128], f16)
                nc.sync.dma_start(bghn0, bghn0_d[:, :, :])
                bgin0 = wpool.tile([1, 2, 128], f16)
                nc.sync.dma_start(bgin0, bgin0_d[:, :, :])
                brz1 = wpool.tile([1, 4, 128], f16)
                nc.sync.dma_start(brz1, brz1_d[:, :, :])
                bghn1 = wpool.tile([1, 2, 128], f16)
                nc.sync.dma_start(bghn1, bghn1_d[:, :, :])
                bgin1 = wpool.tile([1, 2, 128], f16)
                nc.sync.dma_start(bgin1, bgin1_d[:, :, :])
                eb1T = wpool.tile([128, K, 4], f32)
                nc.sync.dma_start(eb1T, eb1T_d[:, :, :])
                eb2T = wpool.tile([128, K, 2], f32)
                nc.sync.dma_start(eb2T, eb2T_d[:, :, :])

            ones1 = wpool.tile([1, 128], f16)
            nc.vector.memset(ones1, 1.0)
            onesK = wpool.tile([128, 1], f16)
            nc.vector.memset(onesK, 1.0)
            ones8 = wpool.tile([1, K], f16)
            nc.vector.memset(ones8, 1.0)
            ident = wpool.tile([128, 128], f16)
            make_identity(nc, ident)

            # initial hidden states (zero)
            h_prev = hpool.tile([128, 2, 128], f16, tag="h0")
            nc.vector.memset(h_prev, 0.0)
            s_prev = hpool.tile([128, 2, 128], f16, tag="h1")
            nc.vector.memset(s_prev, 0.0)

            xT_r = xT_d.rearrange("t i b -> i t b")
            xc = None

            AL = mybir.AluOpType

            def gru_step(wT, ltag, x_in, x_nk, hp, lb):
                """Full GRU cell step: contiguous per-chunk PSUM groups
                (r0, r1, ghn, z0, z1, gin — r first so sigmoid(r) starts
                earliest), then pointwise. Update tail uses zc = 1-z and
                m2 = z*h computed off-chain on GpSimd so the post-tanh
                chain is only mul+add on DVE."""
                brz, bghn, bgin = lb if has_bias else (None, None, None)
                ps_r = psum.tile([128, 2, 128], f32, tag=ltag + "r")
                ps_z = psum.tile([128, 2, 128], f32, tag=ltag + "z")
                ps_gh = psum.tile([128, 2, 128], f32, tag=ltag + "gh")
                ps_gi = psum.tile([128, 2, 128], f32, tag=ltag + "gi")

                def rz_chunk(ps, mb, m):
                    # x/h0-side first (ready early), h-side last so the
                    # queue drains everything possible before h arrives
                    if has_bias:
                        nc.tensor.matmul(
                            ps[:, m], brz[0:1, mb, :], ones1,
                            start=True, stop=False,
                        )
                    for c in range(x_nk):
                        nc.tensor.matmul(
                            ps[:, m],
                            wT[:, 2 + c, mb * 128 : (mb + 1) * 128],
                            x_in[c],
                            start=(c == 0 and not has_bias), stop=False,
                        )
                    for c in range(2):
                        nc.tensor.matmul(
                            ps[:, m],
                            wT[:, c, mb * 128 : (mb + 1) * 128],
                            hp[:, c, :],
                            start=False, stop=(c == 1),
                        )

                rz_chunk(ps_r, 0, 0)
                rz_chunk(ps_r, 1, 1)
                for m in range(2):  # ghn
                    g = 2 * H + m * 128
                    if has_bias:
                        nc.tensor.matmul(
                            ps_gh[:, m], bghn[0:1, m, :], ones1,
                            start=True, stop=False,
                        )
                    for c in range(2):
                        nc.tensor.matmul(
                            ps_gh[:, m],
                            wT[:, c, g : g + 128],
                            hp[:, c, :],
                            start=(c == 0 and not has_bias), stop=(c == 1),
                        )
                rz_chunk(ps_z, 2, 0)
                rz_chunk(ps_z, 3, 1)
                for m in range(2):  # gin
                    g = 2 * H + m * 128
                    if has_bias:
                        nc.tensor.matmul(
                            ps_gi[:, m], bgin[0:1, m, :], ones1,
                            start=True, stop=False,
                        )
                    for c in range(x_nk):
                        nc.tensor.matmul(
                            ps_gi[:, m],
                            wT[:, 2 + c, g : g + 128],
                            x_in[c],
                            start=(c == 0 and not has_bias),
                            stop=(c == x_nk - 1),
                        )

                # pointwise
                sig_r = gpool.tile([128, 2, 128], f16, tag=ltag + "sigr")
                nc.scalar.activation(sig_r, ps_r, AF.Sigmoid)
                t1 = gpool.tile([128, 2, 128], f16, tag=ltag + "t1")
                nc.vector.tensor_mul(t1, sig_r, ps_gh)
                sig_z = gpool.tile([128, 2, 128], f16, tag=ltag + "sigz")
                nc.scalar.activation(sig_z, ps_z, AF.Sigmoid)
                zc = gpool.tile([128, 2, 128], f16, tag=ltag + "zc")
                nc.gpsimd.tensor_scalar(
                    zc, sig_z, -1.0, 1.0, op0=AL.mult, op1=AL.add
                )
                m2 = gpool.tile([128, 2, 128], f16, tag=ltag + "m2")
                nc.gpsimd.tensor_mul(m2, sig_z, hp)
                t2 = gpool.tile([128, 2, 128], f16, tag=ltag + "t2")
                nc.vector.tensor_add(t2, t1, ps_gi)
                n_t = gpool.tile([128, 2, 128], f16, tag=ltag + "nt")
                nc.scalar.activation(n_t, t2, AF.Tanh)
                m1 = gpool.tile([128, 2, 128], f16, tag=ltag + "m1")
                nc.vector.tensor_mul(m1, n_t, zc)
                h_new = hpool.tile([128, 2, 128], f16, tag=ltag + "h")
                nc.vector.tensor_add(h_new, m1, m2)
                return h_new

            lbias0 = (brz0, bghn0, bgin0) if has_bias else None
            lbias1 = (brz1, bghn1, bgin1) if has_bias else None
            expert_w = {}

            def load_expert_weights():
                expert_w["w1T"] = wpool.tile([128, K, 2, E1], f16, name="w1Tw")
                nc.sync.dma_start(
                    expert_w["w1T"],
                    w1T_d.rearrange("k (c p) e -> p k c e", p=128),
                )
                expert_w["w2T"] = wpool.tile([128, K, 4, E2], f16, name="w2Tw")
                nc.sync.dma_start(
                    expert_w["w2T"],
                    w2T_d.rearrange("k (c p) e -> p k c e", p=128),
                )
                expert_w["w3T"] = wpool.tile([128, K, 2, C], f16, name="w3Tw")
                nc.sync.dma_start(
                    expert_w["w3T"],
                    w3T_d.rearrange("k (c p) e -> p k c e", p=128),
                )
                expert_w["cm2T"] = wpool.tile([128, 2, K], f16, name="cm2Tw")
                nc.sync.dma_start(
                    expert_w["cm2T"], cm2T_d.rearrange("(c p) k -> p c k", p=128)
                )
                expert_w["cc"] = wpool.tile([K, H], f32, name="ccw")
                nc.sync.dma_start(expert_w["cc"], cc_d[:, :])
                expert_w["eb3"] = wpool.tile([1, K, C], f16, name="eb3w")
                nc.sync.dma_start(expert_w["eb3"], eb3_d[:, :, :])

            h0_hist = [None] * tsteps
            for t in range(tsteps):
                if t % XCHUNK == 0:
                    ch = min(XCHUNK, tsteps - t)
                    xc = xpool.tile([128, XCHUNK, BC], f16, tag="xc")
                    nc.sync.dma_start(xc[:, :ch, :], xT_r[:, t : t + ch, :])
                x_t = xc[:, t % XCHUNK, :]
                if t >= 1:
                    hh = h0_hist[t - 1]
                    s_prev = gru_step(
                        wT1, "l1", [hh[:, 0, :], hh[:, 1, :]], 2, s_prev,
                        lbias1,
                    )
                h_prev = gru_step(wT0, "l0", [x_t], 1, h_prev, lbias0)
                h0_hist[t] = h_prev
                if t == 0:
                    load_expert_weights()
            hh = h0_hist[tsteps - 1]
            s_prev = gru_step(
                wT1, "l1", [hh[:, 0, :], hh[:, 1, :]], 2, s_prev, lbias1
            )
            zT = s_prev  # [128, 2, 128] latent, H on partitions (folded)

            # ---- soft cluster assignment q (Student-t, alpha=1) ----
            # d2[k,b] = |z_b|^2 - 2 c_k . z_b + |c_k|^2 ;  q = 1/(1+d2), norm.
            zsq = gpool.tile([128, 2, 128], f16, tag="zsq")
            nc.vector.tensor_mul(zsq, zT, zT)
            ps_z2 = psum.tile([1, 128], f32, tag="l1z")
            for c in range(2):  # |z|^2 row
                nc.tensor.matmul(
                    ps_z2, onesK, zsq[:, c, :],
                    start=(c == 0), stop=(c == 1),
                )
            z2sb = gpool.tile([1, 128], f16, tag="z2sb")
            nc.vector.tensor_copy(z2sb, ps_z2)
            ps_q = psum.tile([K, 128], f32, tag="l0gh")
            for c in range(2):  # -2 z . c_k
                nc.tensor.matmul(
                    ps_q, expert_w["cm2T"][:, c, :], zT[:, c, :],
                    start=(c == 0), stop=False,
                )
            nc.tensor.matmul(  # + |z|^2 broadcast over k
                ps_q, ones8, z2sb, start=False, stop=True,
            )
            # c2 = |c_k|^2 + 1
            ccsq = gpool.tile([K, H], f32, tag="ccsq")
            nc.vector.tensor_mul(ccsq, expert_w["cc"], expert_w["cc"])
            c2 = gpool.tile([K, 1], f32, tag="c2")
            nc.vector.reduce_sum(c2, ccsq, axis=mybir.AxisListType.X)
            nc.vector.tensor_scalar_add(c2, c2, 1.0)
            d2f = gpool.tile([K, 128], f32, tag="d2f")
            nc.vector.tensor_scalar_add(d2f, ps_q, c2)
            qun = gpool.tile([K, 128], f16, tag="qun")
            with nc.allow_low_precision(reason="q weights are O(1e-2); fp16 ample"):
                nc.vector.reciprocal(qun, d2f)
            ps_qT = psum.tile([128, K], f16, tag="l0gi")
            nc.tensor.transpose(ps_qT, qun, ident[0:K, 0:K])
            qTs = gpool.tile([128, K], f16, tag="qTs")
            nc.vector.tensor_copy(qTs, ps_qT)
            qsum = gpool.tile([128, 1], f32, tag="qsum")
            nc.vector.reduce_sum(qsum, qTs, axis=mybir.AxisListType.X)
            rq = gpool.tile([128, 1], f32, tag="rq")
            nc.vector.reciprocal(rq, qsum)

            # ---- experts (all 8 on this core's batch shard) ----
            # software-pipelined over k so MM1(k+1) sits ahead of MM2(k)
            # in the PE queue (no head-of-line block on relu)
            ps_out = psum.tile([128, K, C], f32, tag="l1gh")
            e1ps = [None] * K
            e2ps = [None] * K
            h1ss = [None] * K
            h2ss = [None] * K

            def e_mm1(k):
                ps_e1 = psum.tile(
                    [128, 4, 128], f32, tag=("l0r" if k % 2 == 0 else "l0z"),
                    name=f"pse1_{k}",
                )
                e1ps[k] = ps_e1
                for m in range(4):
                    for c in range(2):
                        nc.tensor.matmul(
                            ps_e1[:, m],
                            expert_w["w1T"][:, k, c, m * 128 : (m + 1) * 128],
                            zT[:, c, :],
                            start=(c == 0), stop=(c == 1),
                        )

            def e_relu1(k):
                h1s = gpool.tile([128, 4, 128], f16, tag="l0sig", name=f"h1s_{k}")
                h1ss[k] = h1s
                if has_bias:
                    for m in range(4):
                        nc.scalar.activation(
                            h1s[:, m, :], e1ps[k][:, m], AF.Relu,
                            bias=eb1T[:, k, m : m + 1],
                        )
                else:
                    nc.scalar.activation(h1s[:, 0:2, :], e1ps[k][:, 0:2], AF.Relu)
                    nc.scalar.activation(h1s[:, 2:4, :], e1ps[k][:, 2:4], AF.Relu)

            def e_mm2(k):
                ps_e2 = psum.tile(
                    [128, 2, 128], f32, tag=("l1r" if k % 2 == 0 else "l1z"),
                    name=f"pse2_{k}",
                )
                e2ps[k] = ps_e2
                for m in range(2):
                    for c in range(4):
                        nc.tensor.matmul(
                            ps_e2[:, m],
                            expert_w["w2T"][:, k, c, m * 128 : (m + 1) * 128],
                            h1ss[k][:, c, :],
                            start=(c == 0), stop=(c == 3),
                        )

            def e_relu2(k):
                h2s = gpool.tile([128, 2, 128], f16, tag="l1sig", name=f"h2s_{k}")
                h2ss[k] = h2s
                if has_bias:
                    for m in range(2):
                        nc.scalar.activation(
                            h2s[:, m, :], e2ps[k][:, m], AF.Relu,
                            bias=eb2T[:, k, m : m + 1],
                        )
                else:
                    nc.scalar.activation(h2s, e2ps[k], AF.Relu)

            def e_mm3(k):
                nc.tensor.matmul(
                    ps_out[:, k, :], ones1, expert_w["eb3"][0:1, k, :],
                    start=True, stop=False,
                )
                for c in range(2):
                    nc.tensor.matmul(
                        ps_out[:, k, :],
                        h2ss[k][:, c, :],
                        expert_w["w3T"][:, k, c, :],
                        start=False, stop=(c == 1),
                    )

            for k in range(K + 2):
                if k < K:
                    e_mm1(k)
                    e_relu1(k)
                if 1 <= k:
                    if k - 1 < K:
                        e_mm2(k - 1)
                        e_relu2(k - 1)
                if 2 <= k:
                    e_mm3(k - 2)

            # ---- q-weighted combine (batch-major) ----
            lgB = gpool.tile([128, K, C], f16, tag="lgB")
            nc.vector.tensor_copy(lgB, ps_out)
            pr_un = gpool.tile([128, C], f32, tag="prun")
            for c in range(C):
                tmpc = gpool.tile([128, K], f32, tag="tmpc")
                nc.vector.tensor_mul(tmpc, lgB[:, :, c], qTs)
                nc.vector.reduce_sum(
                    pr_un[:, c : c + 1], tmpc, axis=mybir.AxisListType.X
                )
            pr = gpool.tile([128, C], f32, tag="pr")
            nc.vector.tensor_scalar_mul(pr, pr_un, rq)
            nc.sync.dma_start(preds_d[:, :], pr)

    return nc


def _prep_core_inputs(inputs, has_bias):
    """Host-side repack: transposed fp16 weights (shared) + per-core xT."""
    f = np.float16
    shared = {}
    shared["wT0"] = np.ascontiguousarray(
        np.concatenate([inputs["W_hh0"], inputs["W_ih0"]], axis=1).T
    ).astype(f)
    shared["wT1"] = np.ascontiguousarray(
        np.concatenate([inputs["W_hh1"], inputs["W_ih1"]], axis=1).T
    ).astype(f)
    shared["w1T"] = np.ascontiguousarray(
        inputs["eW1"].transpose(0, 2, 1)
    ).astype(f)
    shared["w2T"] = np.ascontiguousarray(
        inputs["eW2"].transpose(0, 2, 1)
    ).astype(f)
    shared["w3T"] = np.ascontiguousarray(
        inputs["eW3"].transpose(0, 2, 1)
    ).astype(f)
    ccf = np.asarray(inputs["cluster_centers"], np.float32)
    shared["cm2T"] = np.ascontiguousarray((-2.0 * ccf).T).astype(f)
    shared["cc"] = np.ascontiguousarray(ccf)
    shared["eb3"] = np.asarray(inputs["eb3"], np.float32).reshape(1, K, C).astype(f)
    if has_bias:
        bi0, bh0 = np.asarray(inputs["b_ih0"]), np.asarray(inputs["b_hh0"])
        bi1, bh1 = np.asarray(inputs["b_ih1"]), np.asarray(inputs["b_hh1"])
        shared["brz0"] = (bi0 + bh0)[: 2 * H].reshape(1, 4, 128).astype(f)
        shared["bghn0"] = bh0[2 * H :].reshape(1, 2, 128).astype(f)
        shared["bgin0"] = bi0[2 * H :].reshape(1, 2, 128).astype(f)
        shared["brz1"] = (bi1 + bh1)[: 2 * H].reshape(1, 4, 128).astype(f)
        shared["bghn1"] = bh1[2 * H :].reshape(1, 2, 128).astype(f)
        shared["bgin1"] = bi1[2 * H :].reshape(1, 2, 128).astype(f)
        shared["eb1T"] = np.ascontiguousarray(
            np.asarray(inputs["eb1"], np.float32).reshape(K, 4, 128).transpose(2, 0, 1)
        )
        shared["eb2T"] = np.ascontiguousarray(
            np.asarray(inputs["eb2"], np.float32).reshape(K, 2, 128).transpose(2, 0, 1)
        )

    x = np.asarray(inputs["x"], np.float32)
    in_maps = []
    for c in range(NCORES):
        m = dict(shared)
        xc = x[c * BC : (c + 1) * BC]  # [BC, T, I]
        m["xT"] = np.ascontiguousarray(xc.transpose(1, 2, 0)).astype(f)
        in_maps.append(m)
    return in_maps


def kernel(**inputs):
    global LAST_RESULTS
    has_bias = any(
        np.any(np.asarray(inputs[k]))
        for k in ("b_ih0", "b_hh0", "b_ih1", "b_hh1", "eb1", "eb2")
    )
    key = has_bias
    if key not in _NC_CACHE:
        nc = _build(has_bias)
        _hoist_excess_waits(nc)
        _NC_CACHE[key] = nc
    nc = _NC_CACHE[key]
    in_maps = _prep_core_inputs(inputs, has_bias)
    trace = bool(int(os.environ.get("KERNEL_TRACE", "0")))
    res = run_bass_kernel_spmd(
        nc, in_maps, core_ids=list(range(NCORES)), trace=trace
    )
    LAST_RESULTS = res
    out = np.concatenate([r["preds"] for r in res.results], axis=0)
    return out.astype(np.float32)


# revision 27
# speedup vs baseline: 1.3034x; 1.3034x over previous
"""Trainium2 Bass kernel for nn_ExpertNet_GRU (2-layer GRU encoder -> soft
cluster routing -> 8 expert MLPs -> q-weighted combine).

Sharding: data-parallel over batch B=1024 across 8 cores (128 rows/core).
GRU weights + expert weights replicated; no collectives. Each core computes
its own [128, 2] slice of preds; host concatenates.

Layout ("option A", fully transposed): activations live as [feature-on-
partition, batch-on-free] fp16 tiles; weight tiles are stationary lhsT
[K=128, M=128] slices of W.T; gate chunks are folded along the free dim so
pointwise ops run as single [128, 512]/[128, 256] instructions.
"""

import os
import sys

import numpy as np

sys.path.insert(0, "/opt/trn_rl_repo")

import concourse.bass as bass  # noqa: E402
import concourse.tile as tile  # noqa: E402
from concourse import mybir  # noqa: E402
from concourse.bass_utils import run_bass_kernel_spmd  # noqa: E402
from concourse.masks import make_identity  # noqa: E402

B, T, I, H, K = 1024, 128, 128, 256, 8
E1, E2, C = 512, 256, 2
NCORES = 8
BC = B // NCORES  # 128 batch rows per core
XCHUNK = 16  # timesteps per x DMA chunk

f16 = mybir.dt.float16
f32 = mybir.dt.float32
AF = mybir.ActivationFunctionType

_HOIST_UID = [0]


def _hoist_excess_waits(nc):
    """walrus (neuronxcc) in this container allows very few sync-wait slots
    per compute instruction (1 for TT/ACT/TensorScalar). Tile attaches up to
    ~5. Hoist the excess onto standalone InstEventSemaphore entries directly
    before the instruction on the same engine queue — semantically identical
    for monotonic sem-ge waits (engine blocks at the hoisted wait first)."""
    for fn in nc.m.functions:
        for blk in fn.blocks:
            il = blk.instructions
            out = []
            changed = False
            for ins in il:
                keep = 1
                si = ins.sync_info
                if si is not None and si.on_wait and len(si.on_wait) > keep:
                    upd_ids = {u.id for u in si.on_update}
                    waits = list(si.on_wait)
                    hoistable = [
                        w for w in waits
                        if w.sync_type == "semaphore"
                        and w.wait_mode == "sem-ge-imm"
                        and w.wait_reg is None
                        and w.id not in upd_ids
                    ]
                    n_excess = len(waits) - keep
                    excess = hoistable[:n_excess]
                    if excess:
                        kept = [w for w in waits if w not in excess]
                        for w in excess:
                            h = mybir.InstEventSemaphore(
                                name=f"hoistw-{_HOIST_UID[0]}"
                            )
                            _HOIST_UID[0] += 1
                            h.engine = ins.engine
                            h.sync_info = mybir.SyncInfo(
                                on_wait=[w], on_update=[]
                            )
                            out.append(h)
                        ins.sync_info = mybir.SyncInfo(
                            on_wait=kept, on_update=list(si.on_update)
                        )
                        changed = True
                out.append(ins)
            if changed:
                il[:] = out
    return nc


_NC_CACHE = {}
LAST_RESULTS = None


def _build(has_bias: bool, t_steps: int = T):
    nc = bass.Bass()
    tsteps = t_steps

    # ---- DRAM I/O (per core) ----
    xT_d = nc.dram_tensor("xT", [tsteps, I, BC], f16, kind="ExternalInput")
    wT0_d = nc.dram_tensor("wT0", [H + I, 3 * H], f16, kind="ExternalInput")
    wT1_d = nc.dram_tensor("wT1", [2 * H, 3 * H], f16, kind="ExternalInput")
    w1T_d = nc.dram_tensor("w1T", [K, H, E1], f16, kind="ExternalInput")
    w2T_d = nc.dram_tensor("w2T", [K, E1, E2], f16, kind="ExternalInput")
    w3T_d = nc.dram_tensor("w3T", [K, E2, C], f16, kind="ExternalInput")
    cm2T_d = nc.dram_tensor("cm2T", [H, K], f16, kind="ExternalInput")
    cc_d = nc.dram_tensor("cc", [K, H], f32, kind="ExternalInput")
    eb3_d = nc.dram_tensor("eb3", [1, K, C], f16, kind="ExternalInput")
    if has_bias:
        brz0_d = nc.dram_tensor("brz0", [1, 4, 128], f16, kind="ExternalInput")
        bghn0_d = nc.dram_tensor("bghn0", [1, 2, 128], f16, kind="ExternalInput")
        bgin0_d = nc.dram_tensor("bgin0", [1, 2, 128], f16, kind="ExternalInput")
        brz1_d = nc.dram_tensor("brz1", [1, 4, 128], f16, kind="ExternalInput")
        bghn1_d = nc.dram_tensor("bghn1", [1, 2, 128], f16, kind="ExternalInput")
        bgin1_d = nc.dram_tensor("bgin1", [1, 2, 128], f16, kind="ExternalInput")
        eb1T_d = nc.dram_tensor("eb1T", [128, K, 4], f32, kind="ExternalInput")
        eb2T_d = nc.dram_tensor("eb2T", [128, K, 2], f32, kind="ExternalInput")
    preds_d = nc.dram_tensor("preds", [BC, C], f32, kind="ExternalOutput")

    with tile.TileContext(nc) as tc:
        with (
            tc.tile_pool(name="wpool", bufs=1) as wpool,
            tc.tile_pool(name="xpool", bufs=2) as xpool,
            tc.tile_pool(name="hpool", bufs=3) as hpool,
            tc.tile_pool(name="gpool", bufs=2) as gpool,
            tc.tile_pool(name="psum", bufs=1, space="PSUM") as psum,
        ):
            # ---- load constants ----
            wT0 = wpool.tile([128, 3, 3 * H], f16)  # [p, kchunk, gates]
            nc.sync.dma_start(wT0, wT0_d.rearrange("(k p) g -> p k g", p=128))
            wT1 = wpool.tile([128, 4, 3 * H], f16)
            nc.sync.dma_start(wT1, wT1_d.rearrange("(k p) g -> p k g", p=128))
            if has_bias:
                brz0 = wpool.tile([1, 4, 128], f16)
                nc.sync.dma_start(brz0, brz0_d[:, :, :])
                bghn0 = wpool.tile([1, 2, 128], f16)
                nc.sync.dma_start(bghn0, bghn0_d[:, :, :])
                bgin0 = wpool.tile([1, 2, 128], f16)
                nc.sync.dma_start(bgin0, bgin0_d[:, :, :])
                brz1 = wpool.tile([1, 4, 128], f16)
                nc.sync.dma_start(brz1, brz1_d[:, :, :])
                bghn1 = wpool.tile([1, 2, 128], f16)
                nc.sync.dma_start(bghn1, bghn1_d[:, :, :])
                bgin1 = wpool.tile([1, 2, 128], f16)
                nc.sync.dma_start(bgin1, bgin1_d[:, :, :])
                eb1T = wpool.tile([128, K, 4], f32)
                nc.sync.dma_start(eb1T, eb1T_d[:, :, :])
                eb2T = wpool.tile([128, K, 2], f32)
                nc.sync.dma_start(eb2T, eb2T_d[:, :, :])

            ones1 = wpool.tile([1, 128], f16)
            nc.vector.memset(ones1, 1.0)
            onesK = wpool.tile([128, 1], f16)
            nc.vector.memset(onesK, 1.0)
            ones8 = wpool.tile([1, K], f16)
            nc.vector.memset(ones8, 1.0)
            ident = wpool.tile([128, 128], f16)
            make_identity(nc, ident)

            # initial hidden states (zero)
            h_prev = hpool.tile([128, 2, 128], f16, tag="h0")
            nc.vector.memset(h_prev, 0.0)
            s_prev = hpool.tile([128, 2, 128], f16, tag="h1")
            nc.vector.memset(s_prev, 0.0)

            xT_r = xT_d.rearrange("t i b -> i t b")
            xc = None

            AL = mybir.AluOpType

            def gru_step(wT, ltag, x_in, x_nk, hp, lb):
                """Full GRU cell step: contiguous per-chunk PSUM groups
                (r0, r1, ghn, z0, z1, gin — r first so sigmoid(r) starts
                earliest), then pointwise. Update tail uses zc = 1-z and
                m2 = z*h computed off-chain on GpSimd so the post-tanh
                chain is only mul+add on DVE."""
                brz, bghn, bgin = lb if has_bias else (None, None, None)
                ps_r = psum.tile([128, 2, 128], f32, tag=ltag + "r")
                ps_z = psum.tile([128, 2, 128], f32, tag=ltag + "z")
                ps_gh = psum.tile([128, 2, 128], f32, tag=ltag + "gh")
                ps_gi = psum.tile([128, 2, 128], f32, tag=ltag + "gi")

                def rz_chunk(ps, mb, m):
                    # x/h0-side first (ready early), h-side last so the
                    # queue drains everything possible before h arrives
                    if has_bias:
                        nc.tensor.matmul(
                            ps[:, m], brz[0:1, mb, :], ones1,
                            start=True, stop=False,
                        )
                    for c in range(x_nk):
                        nc.tensor.matmul(
                            ps[:, m],
                            wT[:, 2 + c, mb * 128 : (mb + 1) * 128],
                            x_in[c],
                            start=(c == 0 and not has_bias), stop=False,
                        )
                    for c in range(2):
                        nc.tensor.matmul(
                            ps[:, m],
                            wT[:, c, mb * 128 : (mb + 1) * 128],
                            hp[:, c, :],
                            start=False, stop=(c == 1),
                        )

                rz_chunk(ps_r, 0, 0)
                rz_chunk(ps_r, 1, 1)
                for m in range(2):  # ghn
                    g = 2 * H + m * 128
                    if has_bias:
                        nc.tensor.matmul(
                            ps_gh[:, m], bghn[0:1, m, :], ones1,
                            start=True, stop=False,
                        )
                    for c in range(2):
                        nc.tensor.matmul(
                            ps_gh[:, m],
                            wT[:, c, g : g + 128],
                            hp[:, c, :],
                            start=(c == 0 and not has_bias), stop=(c == 1),
                        )
                rz_chunk(ps_z, 2, 0)
                rz_chunk(ps_z, 3, 1)
                for m in range(2):  # gin
                    g = 2 * H + m * 128
                    if has_bias:
                        nc.tensor.matmul(
                            ps_gi[:, m], bgin[0:1, m, :], ones1,
                            start=True, stop=False,
                        )
                    for c in range(x_nk):
                        nc.tensor.matmul(
                            ps_gi[:, m],
                            wT[:, 2 + c, g : g + 128],
                            x_in[c],
                            start=(c == 0 and not has_bias),
                            stop=(c == x_nk - 1),
                        )

                # pointwise
                sig_r = gpool.tile([128, 2, 128], f16, tag=ltag + "sigr")
                nc.scalar.activation(sig_r, ps_r, AF.Sigmoid)
                t1 = gpool.tile([128, 2, 128], f16, tag=ltag + "t1")
                nc.vector.tensor_mul(t1, sig_r, ps_gh)
                sig_z = gpool.tile([128, 2, 128], f16, tag=ltag + "sigz")
                nc.scalar.activation(sig_z, ps_z, AF.Sigmoid)
                zc = gpool.tile([128, 2, 128], f16, tag=ltag + "zc")
                nc.gpsimd.tensor_scalar(
                    zc, sig_z, -1.0, 1.0, op0=AL.mult, op1=AL.add
                )
                m2 = gpool.tile([128, 2, 128], f16, tag=ltag + "m2")
                nc.gpsimd.tensor_mul(m2, sig_z, hp)
                t2 = gpool.tile([128, 2, 128], f16, tag=ltag + "t2")
                nc.vector.tensor_add(t2, t1, ps_gi)
                n_t = gpool.tile([128, 2, 128], f16, tag=ltag + "nt")
                nc.scalar.activation(n_t, t2, AF.Tanh)
                m1 = gpool.tile([128, 2, 128], f16, tag=ltag + "m1")
                nc.vector.tensor_mul(m1, n_t, zc)
                h_new = hpool.tile([128, 2, 128], f16, tag=ltag + "h")
                nc.vector.tensor_add(h_new, m1, m2)
                return h_new

            lbias0 = (brz0, bghn0, bgin0) if has_bias else None
            lbias1 = (brz1, bghn1, bgin1) if has_bias else None
            expert_w = {}

            def load_expert_weights():
                expert_w["w1T"] = wpool.tile([128, K, 2, E1], f16, name="w1Tw")
                nc.sync.dma_start(
                    expert_w["w1T"],
                    w1T_d.rearrange("k (c p) e -> p k c e", p=128),
                )
                expert_w["w2T"] = wpool.tile([128, K, 4, E2], f16, name="w2Tw")
                nc.sync.dma_start(
                    expert_w["w2T"],
                    w2T_d.rearrange("k (c p) e -> p k c e", p=128),
                )
                expert_w["w3T"] = wpool.tile([128, K, 2, C], f16, name="w3Tw")
                nc.sync.dma_start(
                    expert_w["w3T"],
                    w3T_d.rearrange("k (c p) e -> p k c e", p=128),
                )
                expert_w["cm2T"] = wpool.tile([128, 2, K], f16, name="cm2Tw")
                nc.sync.dma_start(
                    expert_w["cm2T"], cm2T_d.rearrange("(c p) k -> p c k", p=128)
                )
                expert_w["cc"] = wpool.tile([K, H], f32, name="ccw")
                nc.sync.dma_start(expert_w["cc"], cc_d[:, :])
                expert_w["eb3"] = wpool.tile([1, K, C], f16, name="eb3w")
                nc.sync.dma_start(expert_w["eb3"], eb3_d[:, :, :])

            h0_hist = [None] * tsteps
            for t in range(tsteps):
                if t % XCHUNK == 0:
                    ch = min(XCHUNK, tsteps - t)
                    xc = xpool.tile([128, XCHUNK, BC], f16, tag="xc")
                    nc.sync.dma_start(xc[:, :ch, :], xT_r[:, t : t + ch, :])
                x_t = xc[:, t % XCHUNK, :]
                h_prev = gru_step(wT0, "l0", [x_t], 1, h_prev, lbias0)
                h0_hist[t] = h_prev
                if t == 0:
                    load_expert_weights()
                if t >= 1:
                    hh = h0_hist[t - 1]
                    s_prev = gru_step(
                        wT1, "l1", [hh[:, 0, :], hh[:, 1, :]], 2, s_prev,
                        lbias1,
                    )
            hh = h0_hist[tsteps - 1]
            s_prev = gru_step(
                wT1, "l1", [hh[:, 0, :], hh[:, 1, :]], 2, s_prev, lbias1
            )
            zT = s_prev  # [128, 2, 128] latent, H on partitions (folded)

            # ---- soft cluster assignment q (Student-t, alpha=1) ----
            # d2[k,b] = |z_b|^2 - 2 c_k . z_b + |c_k|^2 ;  q = 1/(1+d2), norm.
            zsq = gpool.tile([128, 2, 128], f16, tag="zsq")
            nc.vector.tensor_mul(zsq, zT, zT)
            ps_z2 = psum.tile([1, 128], f32, tag="l1z")
            for c in range(2):  # |z|^2 row
                nc.tensor.matmul(
                    ps_z2, onesK, zsq[:, c, :],
                    start=(c == 0), stop=(c == 1),
                )
            z2sb = gpool.tile([1, 128], f16, tag="z2sb")
            nc.vector.tensor_copy(z2sb, ps_z2)
            ps_q = psum.tile([K, 128], f32, tag="l0gh")
            for c in range(2):  # -2 z . c_k
                nc.tensor.matmul(
                    ps_q, expert_w["cm2T"][:, c, :], zT[:, c, :],
                    start=(c == 0), stop=False,
                )
            nc.tensor.matmul(  # + |z|^2 broadcast over k
                ps_q, ones8, z2sb, start=False, stop=True,
            )
            # c2 = |c_k|^2 + 1
            ccsq = gpool.tile([K, H], f32, tag="ccsq")
            nc.vector.tensor_mul(ccsq, expert_w["cc"], expert_w["cc"])
            c2 = gpool.tile([K, 1], f32, tag="c2")
            nc.vector.reduce_sum(c2, ccsq, axis=mybir.AxisListType.X)
            nc.vector.tensor_scalar_add(c2, c2, 1.0)
            d2f = gpool.tile([K, 128], f32, tag="d2f")
            nc.vector.tensor_scalar_add(d2f, ps_q, c2)
            qun = gpool.tile([K, 128], f16, tag="qun")
            with nc.allow_low_precision(reason="q weights are O(1e-2); fp16 ample"):
                nc.vector.reciprocal(qun, d2f)
            ps_qT = psum.tile([128, K], f16, tag="l0gi")
            nc.tensor.transpose(ps_qT, qun, ident[0:K, 0:K])
            qTs = gpool.tile([128, K], f16, tag="qTs")
            nc.vector.tensor_copy(qTs, ps_qT)
            qsum = gpool.tile([128, 1], f32, tag="qsum")
            nc.vector.reduce_sum(qsum, qTs, axis=mybir.AxisListType.X)
            rq = gpool.tile([128, 1], f32, tag="rq")
            nc.vector.reciprocal(rq, qsum)

            # ---- experts (all 8 on this core's batch shard) ----
            # software-pipelined over k so MM1(k+1) sits ahead of MM2(k)
            # in the PE queue (no head-of-line block on relu)
            ps_out = psum.tile([128, K, C], f32, tag="l1gh")
            e1ps = [None] * K
            e2ps = [None] * K
            h1ss = [None] * K
            h2ss = [None] * K

            def e_mm1(k):
                ps_e1 = psum.tile(
                    [128, 4, 128], f32, tag=("l0r" if k % 2 == 0 else "l0z"),
                    name=f"pse1_{k}",
                )
                e1ps[k] = ps_e1
                for m in range(4):
                    for c in range(2):
                        nc.tensor.matmul(
                            ps_e1[:, m],
                            expert_w["w1T"][:, k, c, m * 128 : (m + 1) * 128],
                            zT[:, c, :],
                            start=(c == 0), stop=(c == 1),
                        )

            def e_relu1(k):
                h1s = gpool.tile([128, 4, 128], f16, tag="l0sig", name=f"h1s_{k}")
                h1ss[k] = h1s
                if has_bias:
                    for m in range(4):
                        nc.scalar.activation(
                            h1s[:, m, :], e1ps[k][:, m], AF.Relu,
                            bias=eb1T[:, k, m : m + 1],
                        )
                else:
                    nc.scalar.activation(h1s[:, 0:2, :], e1ps[k][:, 0:2], AF.Relu)
                    nc.scalar.activation(h1s[:, 2:4, :], e1ps[k][:, 2:4], AF.Relu)

            def e_mm2(k):
                ps_e2 = psum.tile(
                    [128, 2, 128], f32, tag=("l1r" if k % 2 == 0 else "l1z"),
                    name=f"pse2_{k}",
                )
                e2ps[k] = ps_e2
                for m in range(2):
                    for c in range(4):
                        nc.tensor.matmul(
                            ps_e2[:, m],
                            expert_w["w2T"][:, k, c, m * 128 : (m + 1) * 128],
                            h1ss[k][:, c, :],
                            start=(c == 0), stop=(c == 3),
                        )

            def e_relu2(k):
                h2s = gpool.tile([128, 2, 128], f16, tag="l1sig", name=f"h2s_{k}")
                h2ss[k] = h2s
                if has_bias:
                    for m in range(2):
                        nc.scalar.activation(
                            h2s[:, m, :], e2ps[k][:, m], AF.Relu,
                            bias=eb2T[:, k, m : m + 1],
                        )
                else:
                    nc.scalar.activation(h2s, e2ps[k], AF.Relu)

            def e_mm3(k):
                nc.tensor.matmul(
                    ps_out[:, k, :], ones1, expert_w["eb3"][0:1, k, :],
                    start=True, stop=False,
                )
                for c in range(2):
                    nc.tensor.matmul(
                        ps_out[:, k, :],
                        h2ss[k][:, c, :],
                        expert_w["w3T"][:, k, c, :],
                        start=False, stop=(c == 1),
                    )

            for k in range(K + 2):
                if k < K:
                    e_mm1(k)
                    e_relu1(k)
                if 1 <= k:
                    if k - 1 < K:
                        e_mm2(k - 1)
                        e_relu2(k - 1)
                if 2 <= k:
                    e_mm3(k - 2)

            # ---- q-weighted combine (batch-major) ----
            lgB = gpool.tile([128, K, C], f16, tag="lgB")
            nc.vector.tensor_copy(lgB, ps_out)
            pr_un = gpool.tile([128, C], f32, tag="prun")
            for c in range(C):
                tmpc = gpool.tile([128, K], f32, tag="tmpc")
                nc.vector.tensor_mul(tmpc, lgB[:, :, c], qTs)
                nc.vector.reduce_sum(
                    pr_un[:, c : c + 1], tmpc, axis=mybir.AxisListType.X
                )
            pr = gpool.tile([128, C], f32, tag="pr")
            nc.vector.tensor_scalar_mul(pr, pr_un, rq)
            nc.sync.dma_start(preds_d[:, :], pr)

    return nc


def _prep_core_inputs(inputs, has_bias):
    """Host-side repack: transposed fp16 weights (shared) + per-core xT."""
    f = np.float16
    shared = {}
    shared["wT0"] = np.ascontiguousarray(
        np.concatenate([inputs["W_hh0"], inputs["W_ih0"]], axis=1).T
    ).astype(f)
    shared["wT1"] = np.ascontiguousarray(
        np.concatenate([inputs["W_hh1"], inputs["W_ih1"]], axis=1).T
    ).astype(f)
    shared["w1T"] = np.ascontiguousarray(
        inputs["eW1"].transpose(0, 2, 1)
    ).astype(f)
    shared["w2T"] = np.ascontiguousarray(
        inputs["eW2"].transpose(0, 2, 1)
    ).astype(f)
    shared["w3T"] = np.ascontiguousarray(
        inputs["eW3"].transpose(0, 2, 1)
    ).astype(f)
    ccf = np.asarray(inputs["cluster_centers"], np.float32)
    shared["cm2T"] = np.ascontiguousarray((-2.0 * ccf).T).astype(f)
    shared["cc"] = np.ascontiguousarray(ccf)
    shared["eb3"] = np.asarray(inputs["eb3"], np.float32).reshape(1, K, C).astype(f)
    if has_bias:
        bi0, bh0 = np.asarray(inputs["b_ih0"]), np.asarray(inputs["b_hh0"])
        bi1, bh1 = np.asarray(inputs["b_ih1"]), np.asarray(inputs["b_hh1"])
        shared["brz0"] = (bi0 + bh0)[: 2 * H].reshape(1, 4, 128).astype(f)
        shared["bghn0"] = bh0[2 * H :].reshape(1, 2, 128).astype(f)
        shared["bgin0"] = bi0[2 * H :].reshape(1, 2, 128).astype(f)
        shared["brz1"] = (bi1 + bh1)[: 2 * H].reshape(1, 4, 128).astype(f)
        shared["bghn1"] = bh1[2 * H :].reshape(1, 2, 128).astype(f)
        shared["bgin1"] = bi1[2 * H :].reshape(1, 2, 128).astype(f)
        shared["eb1T"] = np.ascontiguousarray(
            np.asarray(inputs["eb1"], np.float32).reshape(K, 4, 128).transpose(2, 0, 1)
        )
        shared["eb2T"] = np.ascontiguousarray(
            np.asarray(inputs["eb2"], np.float32).reshape(K, 2, 128).transpose(2, 0, 1)
        )

    x = np.asarray(inputs["x"], np.float32)
    in_maps = []
    for c in range(NCORES):
        m = dict(shared)
        xc = x[c * BC : (c + 1) * BC]  # [BC, T, I]
        m["xT"] = np.ascontiguousarray(xc.transpose(1, 2, 0)).astype(f)
        in_maps.append(m)
    return in_maps


def kernel(**inputs):
    global LAST_RESULTS
    has_bias = any(
        np.any(np.asarray(inputs[k]))
        for k in ("b_ih0", "b_hh0", "b_ih1", "b_hh1", "eb1", "eb2")
    )
    key = has_bias
    if key not in _NC_CACHE:
        nc = _build(has_bias)
        _hoist_excess_waits(nc)
        _NC_CACHE[key] = nc
    nc = _NC_CACHE[key]
    in_maps = _prep_core_inputs(inputs, has_bias)
    trace = bool(int(os.environ.get("KERNEL_TRACE", "0")))
    res = run_bass_kernel_spmd(
        nc, in_maps, core_ids=list(range(NCORES)), trace=trace
    )
    LAST_RESULTS = res
    out = np.concatenate([r["preds"] for r in res.results], axis=0)
    return out.astype(np.float32)
